# revision 59
# baseline (speedup 1.0000x reference)
"""Trainium2 Bass kernel for nn_CDC_62646392980082 (GRU-CPC loss_fn).

Contract: kernel(**inputs) takes the FULL unsharded inputs (numpy) and
returns the FULL output (loss, acc) exactly like the jax reference.

Strategy (8 NeuronCores, data-parallel over batch B=256 -> 32/core):
  - Transposed layouts (feature dims on SBUF partitions) so every
    contraction is a clean PE matmul; fp16 matmuls with fp32 PSUM
    accumulate and fp32 gate/softmax arithmetic.
  - Pass A holds the (k,r) pairs with r<=3 so its predictions emit
    during the GRU rollout and its dots/softmax start right after
    step 5; pass B (k=0,1) overlaps the pass-A softmax.
  - Elementwise work is spread over DVE / scalar / gpsimd; negatives
    are folded host-side into per-cell multiplicity counts, dead cells
    get a -60000 logit folded into the bias-correction tensor.
  - Per-core partial sums of (loss, correct) are summed on host.
"""

import sys

if "/opt/trn_rl_repo" not in sys.path:
    sys.path.insert(0, "/opt/trn_rl_repo")

import numpy as np
import ml_dtypes

B, K, R, C, P, H, S = 256, 5, 6, 7, 1280, 256, 64
NCORE = 8
BS = B // NCORE            # 32 images per core
BC = BS * C                # 224 (b, c) columns
PC_N = P // 128            # 10 p-chunks
HC_N = H // 128            # 2 h-chunks
IJ = 49                    # 7x7 cells
# pass A: every pair with r <= 3 (ready mid-GRU); pass B: k=0 and k=1 tail
PAIRS_A = [(1, 0), (2, 0), (2, 1), (2, 2), (2, 3),
           (3, 0), (3, 1), (3, 2), (4, 0), (4, 1)]
PAIRS_B = [(0, 0), (0, 1), (0, 2), (0, 3), (0, 4), (0, 5),
           (1, 1), (1, 2), (1, 3), (1, 4)]
PAIRS = PAIRS_A + PAIRS_B
NPAIR = len(PAIRS)
HALF = 10                  # pairs per pass
N_PREDS = NPAIR * B * C    # 35840 global predictions

_CACHE = {}


def _build_program():
    import concourse.bacc as bacc
    import concourse.mybir as mybir
    from concourse.tile import TileContext

    f32 = mybir.dt.float32
    bf16 = mybir.dt.float16  # fp16: same PE rate as bf16, 4x mantissa
    Alu = mybir.AluOpType
    Act = mybir.ActivationFunctionType

    nc = bacc.Bacc()
    dp = nc.declare_dram_parameter
    encT = dp("encT", [128, PC_N * R * BC], bf16, isOutput=False)   # GRU layout
    encB = dp("encB", [128, PC_N * BS * IJ], bf16, isOutput=False)  # dots layout
    wih = dp("wih", [128, PC_N * 768], bf16, isOutput=False)
    whh = dp("whh", [128, HC_N * 768], bf16, isOutput=False)
    wk = dp("wk", [K, 128, HC_N * P], bf16, isOutput=False)
    brz = dp("brz", [128, 4], f32, isOutput=False)
    bihn = dp("bihn", [128, 2], f32, isOutput=False)
    bhhn = dp("bhhn", [128, 2], f32, isOutput=False)
    wklo = dp("wklo", [128, K * PC_N], f32, isOutput=False)
    wkhi = dp("wkhi", [128, K * PC_N], f32, isOutput=False)
    corr = dp("corr", [70, 2 * BS * IJ], bf16, isOutput=False)
    cnt1 = dp("cnt1", [70, 2 * BS * IJ], bf16, isOutput=False)
    posm = dp("posm", [70, 2 * IJ], f32, isOutput=False)
    out = dp("out", [1, 2], f32, isOutput=True)

    with TileContext(nc, pool_alloc_mode="queue") as tc:
        with tc.tile_pool(name="pers", bufs=1) as pers:
            # ---- persistent small loads ----
            brz_t = pers.tile([128, 4], f32)
            nc.sync.dma_start(out=brz_t, in_=brz[:, :])
            bihn_t = pers.tile([128, 2], f32)
            nc.sync.dma_start(out=bihn_t, in_=bihn[:, :])
            bhhn_t = pers.tile([128, 2], f32)
            nc.sync.dma_start(out=bhhn_t, in_=bhhn[:, :])
            wklo_t = pers.tile([128, K * PC_N], f32)
            nc.sync.dma_start(out=wklo_t, in_=wklo[:, :])
            wkhi_t = pers.tile([128, K * PC_N], f32)
            nc.sync.dma_start(out=wkhi_t, in_=wkhi[:, :])
            whh_b = pers.tile([128, HC_N * 768], bf16, name="whh_b")
            whh_t = [whh_b[:, h * 768 : (h + 1) * 768] for h in range(HC_N)]

            # zero initial hidden state (bf16), same layout as a ctx tile
            zb = pers.tile([128, 896], bf16)
            nc.vector.memset(zb, 0.0)

            # GRU context: one tile per r-pair, cols = hc*448 + (r%2)*224 + x
            # (tightly packed: preds stream 448, gh streams 224, no pads)
            ctxp = [
                pers.tile([128, 896], bf16, tag=f"ctx{rp}", name=f"ctx{rp}")
                for rp in range(R // 2)
            ]

            def ctx_r(h, r):
                off = h * 448 + (r % 2) * 224
                return ctxp[r // 2][:, off : off + 224]

            def ctx_pair(r):
                # [128, hc(2), BC] view of step r across both h-chunks
                return ctxp[r // 2].rearrange("p (h x) -> p h x", h=2)[
                    :, :, (r % 2) * 224 : (r % 2) * 224 + BC
                ]

            def zb_pair():
                return zb.rearrange("p (h x) -> p h x", h=2)[:, :, :BC]

            outS = pers.tile([1, 2], f32)

            # dots-phase inputs (DMAs issued late so the sync queue feeds
            # the GRU front first)
            encB_b = pers.tile([128, PC_N * BS * IJ], bf16, name="encB_b")
            posm_t = pers.tile([70, 2 * IJ], f32)
            cnt1_t = pers.tile([70, 2 * BS * IJ], bf16)
            corr_t = pers.tile([70, 2 * BS * IJ], bf16)

            # pools shared across phases
            ppA = tc.alloc_tile_pool(name="ppA", bufs=1)
            ppD = tc.alloc_tile_pool(name="ppD", bufs=1)
            psPP = tc.alloc_tile_pool(name="psPP", bufs=3, space="PSUM")
            psGH = tc.alloc_tile_pool(name="psGH", bufs=3, space="PSUM")
            predsT = [
                ppA.tile([128, BS * HALF * C], bf16, tag=f"pt{i}", name=f"pt{i}")
                for i in range(PC_N)
            ]

            # ---- dots/post working set ----
            D = ppD.tile([70, 2 * BS * IJ], f32)
            G2 = BS  # groups per half
            mx = ppD.tile([70, 2 * G2], f32, tag="mx")
            se = ppD.tile([70, 2 * G2], f32, tag="se")
            pos = ppD.tile([70, 2 * G2], f32, tag="pos")
            lnv = ppD.tile([70, 2 * G2], f32, tag="lnv")
            corr2 = ppD.tile([70, 2 * G2], f32, tag="corr2")
            Ssum = ppD.tile([70, 10], f32, tag="S")

            predsT_B = None  # second buffer, allocated once phase-1 space frees

            def preds_tiles(pass_i):
                return predsT if pass_i == 0 else predsT_B

            def emit_wk(pass_i, k):
                wkb_big = ppA.tile(
                    [128, HC_N * P], bf16, tag="wkbig", bufs=2,
                    name=f"wk{pass_i}_{k}",
                )
                for s in range(2):
                    sl = slice(64 * s, 64 * s + 64)
                    nc.sync.dma_start(out=wkb_big[sl, :], in_=wk[k, sl, :])
                return [wkb_big[:, hc * P : (hc + 1) * P] for hc in range(HC_N)]

            def emit_preds_chunk(pass_i, k, wk_t, qc, nq, rs):
                for m in range(PC_N):
                    ps = psPP.tile(
                        [128, 448], f32, tag="pp", name=f"pp_{pass_i}_{qc}_{m}"
                    )
                    for hc in range(HC_N):
                        if nq == 2:
                            assert rs[1] == rs[0] + 1 and rs[0] % 2 == 0
                            rhs = ctxp[rs[0] // 2][:, hc * 448 : (hc + 1) * 448]
                        else:
                            rhs = ctx_r(hc, rs[0])
                        nc.tensor.matmul(
                            ps[:, : nq * BC],
                            wk_t[hc][:, m * 128 : (m + 1) * 128],
                            rhs,
                            start=(hc == 0),
                            stop=(hc == HC_N - 1),
                        )
                    psv = ps[:, : nq * BC].rearrange(
                        "p (q b c) -> p q b c", q=nq, b=BS
                    )
                    dst = preds_tiles(pass_i)[m].rearrange(
                        "p (b q c) -> p q b c", b=BS, q=HALF
                    )[:, qc : qc + nq, :, :]
                    lo = wklo_t[:, k * PC_N + m : k * PC_N + m + 1]
                    hi = wkhi_t[:, k * PC_N + m : k * PC_N + m + 1]
                    # back half (pass B) is DVE-bound while scalar idles:
                    # route more PSUM drains through scalar there
                    direct = (m % 2 == 0) if pass_i == 0 else (m % 4 == 0)
                    if direct:
                        # DVE clips straight out of PSUM
                        nc.vector.tensor_scalar(dst, psv, lo, hi, Alu.max, Alu.min)
                    else:
                        # scalar drains PSUM; DVE clips from SBUF
                        at = ppA.tile(
                            [128, nq * BC], f32, tag="at", bufs=2,
                            name=f"at_{pass_i}_{qc}_{m}",
                        )
                        atv = at.rearrange("p (q x) -> p q x", q=nq)
                        nc.scalar.activation(
                            atv,
                            ps[:, : nq * BC].rearrange("p (q x) -> p q x", q=nq),
                            Act.Identity,
                        )
                        src = at.rearrange("p (q b c) -> p q b c", q=nq, b=BS)
                        nc.vector.tensor_scalar(dst, src, lo, hi, Alu.max, Alu.min)

            # dots machinery (psMS pool opens mid-phase-1 once psGI frees)
            psMS = None
            encB_t = [encB_b[:, i * BS * IJ : (i + 1) * BS * IJ] for i in range(PC_N)]

            def emit_dots_pass(pass_i, b0=0, b1=BS):
                # 4-b batched PSUM tiles: one D-add per 4 images
                for bg in range(b0, b1, 4):
                    nb = min(4, b1 - bg)
                    ps = psMS.tile(
                        [70, 4 * IJ], f32, tag="dp", name=f"dp{pass_i}_{bg}"
                    )
                    for bi in range(nb):
                        b = bg + bi
                        for pc in range(PC_N):
                            nc.tensor.matmul(
                                ps[:, bi * IJ : (bi + 1) * IJ],
                                preds_tiles(pass_i)[pc][:, b * 70 : (b + 1) * 70],
                                encB_t[pc][:, b * IJ : (b + 1) * IJ],
                                start=(pc == 0),
                                stop=(pc == PC_N - 1),
                            )
                    gsl = slice(
                        (pass_i * BS + bg) * IJ, (pass_i * BS + bg + nb) * IJ
                    )
                    nc.vector.tensor_tensor(
                        D[:, gsl], ps[:, : nb * IJ], corr_t[:, gsl], op=Alu.add
                    )

            # ---- phase 1: gi = x @ W_ih.T, interleaved with GRU steps ----
            psGI = tc.alloc_tile_pool(name="psGI", bufs=2, space="PSUM")
            with tc.tile_pool(name="p1", bufs=1) as p1:
                enc_b = p1.tile([128, PC_N * R * BC], bf16, name="enc_b")
                wih_b = p1.tile([128, PC_N * 768], bf16, name="wih_b")
                # front-critical DMAs spread across queues (descriptor issue
                # is ~0.7us each on the issuing engine): enc chunk0 on sync,
                # wih on gpsimd, whh + enc tail on scalar
                for i in range(PC_N):
                    wsl = slice(i * 768, (i + 1) * 768)
                    nc.gpsimd.dma_start(out=wih_b[:, wsl], in_=wih[:, wsl])
                    csl = slice(i * R * BC, i * R * BC + 448)
                    nc.sync.dma_start(out=enc_b[:, csl], in_=encT[:, csl])
                nc.scalar.dma_start(out=whh_b, in_=whh[:, :])
                for i in range(PC_N):
                    csl = slice(i * R * BC + 448, i * R * BC + 896)
                    nc.sync.dma_start(out=enc_b[:, csl], in_=encT[:, csl])
                for i in range(PC_N):
                    csl = slice(i * R * BC + 896, (i + 1) * R * BC)
                    nc.scalar.dma_start(out=enc_b[:, csl], in_=encT[:, csl])
                enc_t = [enc_b[:, i * R * BC : (i + 1) * R * BC] for i in range(PC_N)]
                wih_t = [wih_b[:, i * 768 : (i + 1) * 768] for i in range(PC_N)]

                # gate-major gi store: gisG[g] cols = r*448 + t*224 + x,
                # with the per-(gate,t) bias folded in at drain time so the
                # sigmoid/tanh activations need no bias and t-pair into one op
                gisG = [
                    p1.tile([128, R * 448], bf16, tag=f"gis{g}", name=f"gis{g}")
                    for g in range(3)
                ]
                gi_bias = {
                    0: brz_t[:, 0:1], 1: brz_t[:, 1:2],
                    2: brz_t[:, 2:3], 3: brz_t[:, 3:4],
                    4: bihn_t[:, 0:1], 5: bihn_t[:, 1:2],
                }

                def gi_pair(g, r):
                    return gisG[g][:, r * 448 : (r + 1) * 448]

                def emit_gi_chunk(cj):
                    # full 10-pc accumulation in PSUM over a 2-r chunk; one
                    # bias-folding drain per (m, chunk), DVE/scalar alternating
                    for m in range(6):
                        g, t = m // 2, m % 2
                        ps = psGI.tile(
                            [128, 448], f32, tag="gi", name=f"gi_{cj}_{m}"
                        )
                        for pc in range(PC_N):
                            nc.tensor.matmul(
                                ps,
                                wih_t[pc][:, m * 128 : (m + 1) * 128],
                                enc_t[pc][:, cj * 448 : cj * 448 + 448],
                                start=(pc == 0),
                                stop=(pc == PC_N - 1),
                            )
                        src = ps.rearrange("p (r x) -> p r x", r=2)
                        dst = gisG[g].rearrange(
                            "p (r t x) -> p r t x", r=R, t=2
                        )[:, 2 * cj : 2 * cj + 2, t, :]
                        # all drains on scalar: DVE is the phase-1 bottleneck
                        nc.scalar.activation(
                            dst, src, Act.Identity, bias=gi_bias[m]
                        )

                def emit_gru_step(r):
                    hprev = [zb[:, h * 448 : h * 448 + 224] for h in range(HC_N)] \
                        if r == 0 else [ctx_r(h, r - 1) for h in range(HC_N)]
                    hprev_pair = zb_pair() if r == 0 else ctx_pair(r - 1)
                    ghp = []
                    for mp in range(3):
                        ps = psGH.tile([128, 512], f32, tag="gh", name=f"gh_{r}_{mp}")
                        for half in range(2):
                            m = mp * 2 + half
                            for hc in range(HC_N):
                                nc.tensor.matmul(
                                    ps[:, half * 256 : half * 256 + BC],
                                    whh_t[hc][:, m * 128 : (m + 1) * 128],
                                    hprev[hc],
                                    start=(hc == 0),
                                    stop=(hc == HC_N - 1),
                                )
                        ghp.append(ps)

                    def gh_pair(g):
                        # [128, t(2), BC] view over the gate's two t-halves
                        return ghp[g].rearrange("p (t x) -> p t x", t=2)[:, :, :BC]

                    def pv(tile):
                        return tile.rearrange("p (t x) -> p t x", t=2)

                    # t-paired gate math (448 wide)
                    tA = p1.tile([128, 448], f32, tag="tAB", bufs=2, name=f"tA{r}")
                    nc.vector.tensor_tensor(
                        pv(tA), pv(gi_pair(0, r)), gh_pair(0), op=Alu.add
                    )
                    rt = p1.tile([128, 448], bf16, tag="rt", bufs=2, name=f"rt{r}")
                    nc.scalar.activation(rt, tA, Act.Sigmoid)
                    tB = p1.tile([128, 448], f32, tag="tAB", bufs=2, name=f"tB{r}")
                    nc.vector.tensor_tensor(
                        pv(tB), pv(gi_pair(1, r)), gh_pair(1), op=Alu.add
                    )
                    zt = p1.tile([128, 448], bf16, tag="zt", bufs=2, name=f"zt{r}")
                    nc.scalar.activation(zt, tB, Act.Sigmoid)
                    tV = p1.tile([128, 448], f32, tag="tVW", bufs=2, name=f"tV{r}")
                    for t in range(2):
                        nc.vector.scalar_tensor_tensor(
                            tV[:, t * BC : (t + 1) * BC],
                            gh_pair(2)[:, t, :],
                            bhhn_t[:, t : t + 1],
                            rt[:, t * BC : (t + 1) * BC],
                            op0=Alu.add, op1=Alu.mult,
                        )
                    tW = p1.tile([128, 448], f32, tag="tVW", bufs=2, name=f"tW{r}")
                    nc.vector.tensor_tensor(tW, tV, gi_pair(2, r), op=Alu.add)
                    nt = p1.tile([128, 448], bf16, tag="nt", bufs=2, name=f"nt{r}")
                    nc.scalar.activation(nt, tW, Act.Tanh)
                    # SBUF-only fp16 tail on the (otherwise idle) gpsimd
                    tD = p1.tile([128, 448], bf16, tag="tD", bufs=2, name=f"tD{r}")
                    nc.gpsimd.tensor_tensor(pv(tD), hprev_pair, pv(nt), op=Alu.subtract)
                    tE = p1.tile([128, 448], bf16, tag="tE", bufs=2, name=f"tE{r}")
                    nc.gpsimd.tensor_tensor(tE, zt, tD, op=Alu.mult)
                    nc.gpsimd.tensor_tensor(ctx_pair(r), pv(nt), pv(tE), op=Alu.add)

                # ---- schedule: gi + GRU + pass-A preds interleaved ----
                emit_gi_chunk(0)
                emit_gru_step(0)
                emit_gi_chunk(1)
                emit_gru_step(1)
                emit_gi_chunk(2)
                psGI.release()
                wk2 = emit_wk(0, 2)
                emit_preds_chunk(0, 2, wk2, 1, 2, [0, 1])
                emit_gru_step(2)
                wk3 = emit_wk(0, 3)
                emit_preds_chunk(0, 3, wk3, 5, 2, [0, 1])
                emit_gru_step(3)
                emit_preds_chunk(0, 2, wk2, 3, 2, [2, 3])
                emit_preds_chunk(0, 3, wk3, 7, 1, [2])
                wk4 = emit_wk(0, 4)
                emit_preds_chunk(0, 4, wk4, 8, 2, [0, 1])
                emit_gru_step(4)
                wk1 = emit_wk(0, 1)
                emit_preds_chunk(0, 1, wk1, 0, 1, [0])
                emit_gru_step(5)
                # dots-phase DMAs queue behind everything front-critical
                for s in range(4):
                    sl = slice(32 * s, 32 * s + 32)
                    nc.sync.dma_start(out=encB_b[sl, :], in_=encB[sl, :])
                nc.sync.dma_start(out=cnt1_t, in_=cnt1[:, :])
                nc.sync.dma_start(out=corr_t, in_=corr[:, :])
                nc.sync.dma_start(out=posm_t, in_=posm[:, :])

            # ---- phase 3: dots + loss (pass-B preds emitted here too) ----
            psGH.release()
            psMS = tc.alloc_tile_pool(name="psMS", bufs=4, space="PSUM")
            # second preds buffer in the space phase 1 freed: pass-B clips
            # need not wait for pass-A dots to finish reading predsT
            ppB = tc.alloc_tile_pool(name="ppB", bufs=1)
            predsT_B = [
                ppB.tile([128, BS * HALF * C], bf16, tag=f"ptB{i}", name=f"ptB{i}")
                for i in range(PC_N)
            ]

            # post parts: (global group offset, group count, half); the last
            # quarter is split so less softmax work trails the final matmul
            POST_PARTS = [(0, 16, 0), (16, 16, 0), (32, 16, 1),
                          (48, 8, 1), (56, 8, 1)]

            def emit_post_part(pi):
                g0, PG, h = POST_PARTS[pi]
                lo = g0 * IJ
                hi = (g0 + PG) * IJ
                Dh = D[:, lo:hi]
                B2h = ppD.tile([70, PG * IJ], f32, tag="b2", bufs=2, name=f"b2_{pi}")
                Dv = Dh.rearrange("p (g j) -> p g j", j=IJ)
                B2v = B2h.rearrange("p (g j) -> p g j", j=IJ)
                cnt_h = cnt1_t[:, lo:hi]
                gsl = slice(g0, g0 + PG)
                mxh = mx[:, gsl]
                seh = se[:, gsl]
                posh = pos[:, gsl]
                lnvh = lnv[:, gsl]
                corrh = corr2[:, gsl]
                # dead-cell (-60000) mask is folded into corr on host
                nc.vector.tensor_reduce(mxh, Dv, axis=mybir.AxisListType.X, op=Alu.max)
                nc.gpsimd.tensor_tensor(
                    B2v, Dv, mxh.unsqueeze(2).broadcast_to([70, PG, IJ]), op=Alu.subtract
                )
                nc.scalar.activation(B2h, B2h, Act.Exp)
                nc.gpsimd.tensor_tensor(B2h, B2h, cnt_h, op=Alu.mult)
                nc.vector.tensor_reduce(seh, B2v, axis=mybir.AxisListType.X, op=Alu.add)
                # pos = sum(D * posmask) (exact: zeros elsewhere)
                pmh = posm_t[:, h * IJ : (h + 1) * IJ]
                nc.gpsimd.tensor_tensor(
                    B2v, Dv, pmh.unsqueeze(1).broadcast_to([70, PG, IJ]), op=Alu.mult
                )
                nc.vector.tensor_reduce(posh, B2v, axis=mybir.AxisListType.X, op=Alu.add)
                # loss = ln(se) + mx - pos ; correct = (pos >= mx)
                nc.scalar.activation(lnvh, seh, Act.Ln)
                nc.vector.tensor_tensor(lnvh, lnvh, mxh, op=Alu.add)
                nc.vector.tensor_tensor(corrh, posh, mxh, op=Alu.is_ge)
                nc.vector.tensor_tensor(lnvh, lnvh, posh, op=Alu.subtract)
                nc.vector.tensor_reduce(
                    Ssum[:, 2 * pi : 2 * pi + 1], lnvh,
                    axis=mybir.AxisListType.X, op=Alu.add,
                )
                nc.vector.tensor_reduce(
                    Ssum[:, 2 * pi + 1 : 2 * pi + 2], corrh,
                    axis=mybir.AxisListType.X, op=Alu.add,
                )

            # pass-A dots start immediately; pass-B preds + posts interleave
            emit_dots_pass(0)
            wk0 = emit_wk(1, 0)
            emit_preds_chunk(1, 0, wk0, 0, 2, [0, 1])
            emit_preds_chunk(1, 0, wk0, 2, 2, [2, 3])
            emit_post_part(0)
            emit_preds_chunk(1, 0, wk0, 4, 2, [4, 5])
            wk1b = emit_wk(1, 1)
            emit_preds_chunk(1, 1, wk1b, 6, 1, [1])
            emit_preds_chunk(1, 1, wk1b, 7, 1, [2])
            emit_post_part(1)
            emit_preds_chunk(1, 1, wk1b, 8, 1, [3])
            emit_preds_chunk(1, 1, wk1b, 9, 1, [4])
            emit_dots_pass(1, 0, 16)
            emit_post_part(2)
            emit_dots_pass(1, 16, 24)
            emit_post_part(3)
            emit_dots_pass(1, 24, 32)
            emit_post_part(4)

            # combine parts: [loss, acc] = colsums of Ssum pairs
            ones = ppD.tile([70, 1], f32, tag="ones")
            nc.vector.memset(ones, 1.0)
            fp = psMS.tile([1, 10], f32, tag="dp", name="fin")
            nc.tensor.matmul(fp, ones, Ssum, start=True, stop=True)
            fs = ppD.tile([1, 10], f32, tag="fs")
            nc.vector.tensor_copy(fs, fp)
            fs2 = ppD.tile([1, 4], f32, tag="fs2")
            nc.vector.tensor_tensor(fs2[:, 0:2], fs[:, 0:2], fs[:, 2:4], op=Alu.add)
            nc.vector.tensor_tensor(fs2[:, 2:4], fs[:, 4:6], fs[:, 6:8], op=Alu.add)
            fs3 = ppD.tile([1, 2], f32, tag="fs3")
            nc.vector.tensor_tensor(fs3, fs2[:, 0:2], fs2[:, 2:4], op=Alu.add)
            nc.vector.tensor_tensor(outS, fs3, fs[:, 8:10], op=Alu.add)
            nc.sync.dma_start(out=out[:, :], in_=outS)
            psMS.release()
            psPP.release()
            ppB.release()
            ppD.release()
            ppA.release()

    nc.finalize()
    return nc


def _prep_inputs(encodings, hidden, W_ih, W_hh, b_ih, b_hh, Wk_w, Wk_b,
                 neg_rows, neg_cols):
    """Host-side reformat of the full inputs into per-core DMA-clean arrays."""
    bf16 = np.float16
    enc = np.ascontiguousarray(encodings, dtype=np.float32)
    e6 = enc.reshape(NCORE, BS, C, C, PC_N, 128)  # (core, b, i, c, pc, pp)
    # GRU layout: [core, pc, pp, r*BC + b*7 + c], r < 6
    encT = np.ascontiguousarray(
        e6[:, :, :R].transpose(0, 5, 4, 2, 1, 3)   # (core, pp, pc, r, b, c)
    ).reshape(NCORE, 128, PC_N * R * BC).astype(bf16)
    # dots layout: [core, pc, pp, b*49 + i*7 + c]
    encB = np.ascontiguousarray(
        e6.transpose(0, 5, 4, 1, 2, 3)   # (core, pp, pc, b, i, c)
    ).reshape(NCORE, 128, PC_N * BS * IJ).astype(bf16)

    wih = np.ascontiguousarray(
        W_ih.T.reshape(PC_N, 128, 768).transpose(1, 0, 2), dtype=np.float32
    ).reshape(128, PC_N * 768).astype(bf16)
    whh = np.ascontiguousarray(
        W_hh.T.reshape(HC_N, 128, 768).transpose(1, 0, 2), dtype=np.float32
    ).reshape(128, HC_N * 768).astype(bf16)
    wkh = np.ascontiguousarray(
        Wk_w.transpose(0, 2, 1).reshape(K, HC_N, 128, P).transpose(0, 2, 1, 3),
        dtype=np.float32,
    ).reshape(K, 128, HC_N * P).astype(bf16)
    bsum = (b_ih + b_hh).astype(np.float32)
    brz = np.ascontiguousarray(bsum[:512].reshape(4, 128).T)
    bihn = np.ascontiguousarray(b_ih[512:].astype(np.float32).reshape(2, 128).T)
    bhhn = np.ascontiguousarray(b_hh[512:].astype(np.float32).reshape(2, 128).T)
    wkbT = np.ascontiguousarray(
        Wk_b.astype(np.float32).reshape(K, PC_N, 128).transpose(2, 0, 1)
    ).reshape(128, K * PC_N)
    wklo = -1.0 - wkbT
    wkhi = 1.0 - wkbT
    # rank-1 bias correction: corr[k, b, ij] = sum_p Wk_b[k,p] * enc[b,i,j,p]
    corr_k = np.einsum(
        "kp,bijp->kbij", Wk_b.astype(np.float32), enc, optimize=True
    ).reshape(K, B, IJ)
    # expand to device layout [core, row=q*7+c, half, b_local, j] (k by pair)
    corr_dev = np.empty((NCORE, HALF * C, 2, BS, IJ), dtype=np.float32)
    for half in range(2):
        for qq in range(HALF):
            k, _r = PAIRS[half * HALF + qq]
            for c in range(C):
                corr_dev[:, qq * 7 + c, half] = corr_k[k].reshape(NCORE, BS, IJ)

    # negatives -> multiplicity counts over the 49 cells, plus the positive
    neg_idx = (neg_rows.astype(np.int64) * 7 + neg_cols.astype(np.int64))  # [B,K,R,C,63]
    sel = np.stack([neg_idx[:, k, r] for (k, r) in PAIRS], axis=1)  # [B,20,C,63]
    flat = (
        np.arange(B * NPAIR * C, dtype=np.int64)[:, None] * IJ
        + sel.reshape(B * NPAIR * C, S - 1)
    ).ravel()
    cnts = np.bincount(flat, minlength=B * NPAIR * C * IJ).reshape(
        B, NPAIR, C, IJ
    ).astype(np.float32)
    cvec = np.arange(C)
    for pi, (k, r) in enumerate(PAIRS):
        cnts[:, pi, cvec, r * 7 + cvec] += 1.0   # include the positive
    # device layout [core, row=q*7+c, half, b_local, j]
    cnt_dev = np.ascontiguousarray(
        cnts.reshape(NCORE, BS, 2, HALF, C, IJ).transpose(0, 3, 4, 2, 1, 5)
    )  # [core, HALF, C, 2, BS, IJ]
    cnt1 = cnt_dev.reshape(NCORE, HALF * C, 2 * BS * IJ).astype(bf16)
    # dead cells (multiplicity 0) get a large negative logit folded into
    # corr so the device softmax skips the explicit is_eq mask pass
    corr_dev = corr_dev.reshape(NCORE, HALF, C, 2, BS, IJ)
    corr_dev[cnt_dev.reshape(NCORE, HALF, C, 2, BS, IJ) == 0] = -60000.0
    corr_dev = corr_dev.reshape(NCORE, HALF * C, 2 * BS * IJ).astype(bf16)

    posm = np.zeros((HALF * C, 2, IJ), dtype=np.float32)
    for half in range(2):
        for qq in range(HALF):
            k, r = PAIRS[half * HALF + qq]
            for c in range(C):
                posm[qq * 7 + c, half, r * 7 + c] = 1.0
    posm = posm.reshape(HALF * C, 2 * IJ)

    in_maps = []
    for core in range(NCORE):
        in_maps.append(
            {
                "encT": encT[core],
                "encB": encB[core],
                "wih": wih,
                "whh": whh,
                "wk": wkh,
                "brz": brz,
                "bihn": bihn,
                "bhhn": bhhn,
                "wklo": wklo,
                "wkhi": wkhi,
                "corr": corr_dev[core],
                "cnt1": cnt1[core],
                "posm": posm,
            }
        )
    return in_maps


def _get_program():
    if "nc" not in _CACHE:
        _CACHE["nc"] = _build_program()
    return _CACHE["nc"]


def run_on_device(in_maps, trace=False, tmpdir=None):
    from concourse.bass_utils import run_bass_kernel_spmd

    nc = _get_program()
    return run_bass_kernel_spmd(
        nc, in_maps, list(range(NCORE)), trace=trace, tmpdir=tmpdir
    )


def kernel(**inputs):
    in_maps = _prep_inputs(**inputs)
    res = run_on_device(in_maps)
    loss_sum = 0.0
    corr_sum = 0.0
    for core in range(NCORE):
        o = res.results[core]["out"]
        loss_sum += float(o[0, 0])
        corr_sum += float(o[0, 1])
    loss = np.float32(loss_sum / N_PREDS)
    acc = np.float32(corr_sum / N_PREDS)
    return loss, acc


# revision 60
# speedup vs baseline: 1.0030x; 1.0030x over previous
"""Trainium2 Bass kernel for nn_CDC_62646392980082 (GRU-CPC loss_fn).

Contract: kernel(**inputs) takes the FULL unsharded inputs (numpy) and
returns the FULL output (loss, acc) exactly like the jax reference.

Strategy (8 NeuronCores, data-parallel over batch B=256 -> 32/core):
  - Transposed layouts (feature dims on SBUF partitions) so every
    contraction is a clean PE matmul; fp16 matmuls with fp32 PSUM
    accumulate and fp32 gate/softmax arithmetic.
  - Pass A holds the (k,r) pairs with r<=3 so its predictions emit
    during the GRU rollout and its dots/softmax start right after
    step 5; pass B (k=0,1) overlaps the pass-A softmax.
  - Elementwise work is spread over DVE / scalar / gpsimd; negatives
    are folded host-side into per-cell multiplicity counts, dead cells
    get a -60000 logit folded into the bias-correction tensor.
  - Per-core partial sums of (loss, correct) are summed on host.
"""

import sys

if "/opt/trn_rl_repo" not in sys.path:
    sys.path.insert(0, "/opt/trn_rl_repo")

import numpy as np
import ml_dtypes

B, K, R, C, P, H, S = 256, 5, 6, 7, 1280, 256, 64
NCORE = 8
BS = B // NCORE            # 32 images per core
BC = BS * C                # 224 (b, c) columns
PC_N = P // 128            # 10 p-chunks
HC_N = H // 128            # 2 h-chunks
IJ = 49                    # 7x7 cells
# pass A: every pair with r <= 3 (ready mid-GRU); pass B: k=0 and k=1 tail
PAIRS_A = [(1, 0), (2, 0), (2, 1), (2, 2), (2, 3),
           (3, 0), (3, 1), (3, 2), (4, 0), (4, 1)]
PAIRS_B = [(0, 0), (0, 1), (0, 2), (0, 3), (0, 4), (0, 5),
           (1, 1), (1, 2), (1, 3), (1, 4)]
PAIRS = PAIRS_A + PAIRS_B
NPAIR = len(PAIRS)
HALF = 10                  # pairs per pass
N_PREDS = NPAIR * B * C    # 35840 global predictions

_CACHE = {}


def _build_program():
    import concourse.bacc as bacc
    import concourse.mybir as mybir
    from concourse.tile import TileContext

    f32 = mybir.dt.float32
    bf16 = mybir.dt.float16  # fp16: same PE rate as bf16, 4x mantissa
    Alu = mybir.AluOpType
    Act = mybir.ActivationFunctionType

    nc = bacc.Bacc()
    dp = nc.declare_dram_parameter
    encT = dp("encT", [128, PC_N * R * BC], bf16, isOutput=False)   # GRU layout
    encB = dp("encB", [128, PC_N * BS * IJ], bf16, isOutput=False)  # dots layout
    wih = dp("wih", [128, PC_N * 768], bf16, isOutput=False)
    whh = dp("whh", [128, HC_N * 768], bf16, isOutput=False)
    wk = dp("wk", [K, 128, HC_N * P], bf16, isOutput=False)
    brz = dp("brz", [128, 4], f32, isOutput=False)
    bihn = dp("bihn", [128, 2], f32, isOutput=False)
    bhhn = dp("bhhn", [128, 2], f32, isOutput=False)
    wklo = dp("wklo", [128, K * PC_N], f32, isOutput=False)
    wkhi = dp("wkhi", [128, K * PC_N], f32, isOutput=False)
    corr = dp("corr", [70, 2 * BS * IJ], bf16, isOutput=False)
    cnt1 = dp("cnt1", [70, 2 * BS * IJ], bf16, isOutput=False)
    posm = dp("posm", [70, 2 * IJ], f32, isOutput=False)
    out = dp("out", [1, 2], f32, isOutput=True)

    with TileContext(nc, pool_alloc_mode="queue") as tc:
        with tc.tile_pool(name="pers", bufs=1) as pers:
            # ---- persistent small loads ----
            brz_t = pers.tile([128, 4], f32)
            nc.sync.dma_start(out=brz_t, in_=brz[:, :])
            bihn_t = pers.tile([128, 2], f32)
            nc.sync.dma_start(out=bihn_t, in_=bihn[:, :])
            bhhn_t = pers.tile([128, 2], f32)
            nc.sync.dma_start(out=bhhn_t, in_=bhhn[:, :])
            wklo_t = pers.tile([128, K * PC_N], f32)
            nc.sync.dma_start(out=wklo_t, in_=wklo[:, :])
            wkhi_t = pers.tile([128, K * PC_N], f32)
            nc.sync.dma_start(out=wkhi_t, in_=wkhi[:, :])
            whh_b = pers.tile([128, HC_N * 768], bf16, name="whh_b")
            whh_t = [whh_b[:, h * 768 : (h + 1) * 768] for h in range(HC_N)]

            # zero initial hidden state (bf16), same layout as a ctx tile
            zb = pers.tile([128, 896], bf16)
            nc.vector.memset(zb, 0.0)

            # GRU context: one tile per r-pair, cols = hc*448 + (r%2)*224 + x
            # (tightly packed: preds stream 448, gh streams 224, no pads)
            ctxp = [
                pers.tile([128, 896], bf16, tag=f"ctx{rp}", name=f"ctx{rp}")
                for rp in range(R // 2)
            ]

            def ctx_r(h, r):
                off = h * 448 + (r % 2) * 224
                return ctxp[r // 2][:, off : off + 224]

            def ctx_pair(r):
                # [128, hc(2), BC] view of step r across both h-chunks
                return ctxp[r // 2].rearrange("p (h x) -> p h x", h=2)[
                    :, :, (r % 2) * 224 : (r % 2) * 224 + BC
                ]

            def zb_pair():
                return zb.rearrange("p (h x) -> p h x", h=2)[:, :, :BC]

            outS = pers.tile([1, 2], f32)

            # dots-phase inputs (DMAs issued late so the sync queue feeds
            # the GRU front first)
            encB_b = pers.tile([128, PC_N * BS * IJ], bf16, name="encB_b")
            posm_t = pers.tile([70, 2 * IJ], f32)
            cnt1_t = pers.tile([70, 2 * BS * IJ], bf16)
            corr_t = pers.tile([70, 2 * BS * IJ], bf16)

            # pools shared across phases
            ppA = tc.alloc_tile_pool(name="ppA", bufs=1)
            ppD = tc.alloc_tile_pool(name="ppD", bufs=1)
            psPP = tc.alloc_tile_pool(name="psPP", bufs=3, space="PSUM")
            psGH = tc.alloc_tile_pool(name="psGH", bufs=3, space="PSUM")
            predsT = [
                ppA.tile([128, BS * HALF * C], bf16, tag=f"pt{i}", name=f"pt{i}")
                for i in range(PC_N)
            ]

            # ---- dots/post working set ----
            D = ppD.tile([70, 2 * BS * IJ], f32)
            G2 = BS  # groups per half
            mx = ppD.tile([70, 2 * G2], f32, tag="mx")
            se = ppD.tile([70, 2 * G2], f32, tag="se")
            pos = ppD.tile([70, 2 * G2], f32, tag="pos")
            lnv = ppD.tile([70, 2 * G2], f32, tag="lnv")
            corr2 = ppD.tile([70, 2 * G2], f32, tag="corr2")
            Ssum = ppD.tile([70, 10], f32, tag="S")

            predsT_B = None  # second buffer, allocated once phase-1 space frees

            def preds_tiles(pass_i):
                return predsT if pass_i == 0 else predsT_B

            def emit_wk(pass_i, k):
                wkb_big = ppA.tile(
                    [128, HC_N * P], bf16, tag="wkbig", bufs=2,
                    name=f"wk{pass_i}_{k}",
                )
                for s in range(2):
                    sl = slice(64 * s, 64 * s + 64)
                    nc.sync.dma_start(out=wkb_big[sl, :], in_=wk[k, sl, :])
                return [wkb_big[:, hc * P : (hc + 1) * P] for hc in range(HC_N)]

            def emit_preds_chunk(pass_i, k, wk_t, qc, nq, rs):
                for m in range(PC_N):
                    ps = psPP.tile(
                        [128, 448], f32, tag="pp", name=f"pp_{pass_i}_{qc}_{m}"
                    )
                    for hc in range(HC_N):
                        if nq == 2:
                            assert rs[1] == rs[0] + 1 and rs[0] % 2 == 0
                            rhs = ctxp[rs[0] // 2][:, hc * 448 : (hc + 1) * 448]
                        else:
                            rhs = ctx_r(hc, rs[0])
                        nc.tensor.matmul(
                            ps[:, : nq * BC],
                            wk_t[hc][:, m * 128 : (m + 1) * 128],
                            rhs,
                            start=(hc == 0),
                            stop=(hc == HC_N - 1),
                        )
                    psv = ps[:, : nq * BC].rearrange(
                        "p (q b c) -> p q b c", q=nq, b=BS
                    )
                    dst = preds_tiles(pass_i)[m].rearrange(
                        "p (b q c) -> p q b c", b=BS, q=HALF
                    )[:, qc : qc + nq, :, :]
                    lo = wklo_t[:, k * PC_N + m : k * PC_N + m + 1]
                    hi = wkhi_t[:, k * PC_N + m : k * PC_N + m + 1]
                    if m % 2 == 0:
                        # DVE clips straight out of PSUM
                        nc.vector.tensor_scalar(dst, psv, lo, hi, Alu.max, Alu.min)
                    else:
                        # scalar drains PSUM; DVE clips from SBUF
                        at = ppA.tile(
                            [128, nq * BC], f32, tag="at", bufs=2,
                            name=f"at_{pass_i}_{qc}_{m}",
                        )
                        atv = at.rearrange("p (q x) -> p q x", q=nq)
                        nc.scalar.activation(
                            atv,
                            ps[:, : nq * BC].rearrange("p (q x) -> p q x", q=nq),
                            Act.Identity,
                        )
                        src = at.rearrange("p (q b c) -> p q b c", q=nq, b=BS)
                        nc.vector.tensor_scalar(dst, src, lo, hi, Alu.max, Alu.min)

            # dots machinery (psMS pool opens mid-phase-1 once psGI frees)
            psMS = None
            encB_t = [encB_b[:, i * BS * IJ : (i + 1) * BS * IJ] for i in range(PC_N)]

            def emit_dots_pass(pass_i, b0=0, b1=BS):
                # 4-b batched PSUM tiles: one D-add per 4 images
                for bg in range(b0, b1, 4):
                    nb = min(4, b1 - bg)
                    ps = psMS.tile(
                        [70, 4 * IJ], f32, tag="dp", name=f"dp{pass_i}_{bg}"
                    )
                    for bi in range(nb):
                        b = bg + bi
                        for pc in range(PC_N):
                            nc.tensor.matmul(
                                ps[:, bi * IJ : (bi + 1) * IJ],
                                preds_tiles(pass_i)[pc][:, b * 70 : (b + 1) * 70],
                                encB_t[pc][:, b * IJ : (b + 1) * IJ],
                                start=(pc == 0),
                                stop=(pc == PC_N - 1),
                            )
                    gsl = slice(
                        (pass_i * BS + bg) * IJ, (pass_i * BS + bg + nb) * IJ
                    )
                    nc.vector.tensor_tensor(
                        D[:, gsl], ps[:, : nb * IJ], corr_t[:, gsl], op=Alu.add
                    )

            # ---- phase 1: gi = x @ W_ih.T, interleaved with GRU steps ----
            psGI = tc.alloc_tile_pool(name="psGI", bufs=2, space="PSUM")
            with tc.tile_pool(name="p1", bufs=1) as p1:
                enc_b = p1.tile([128, PC_N * R * BC], bf16, name="enc_b")
                wih_b = p1.tile([128, PC_N * 768], bf16, name="wih_b")
                # front-critical DMAs spread across queues (descriptor issue
                # is ~0.7us each on the issuing engine): enc chunk0 on sync,
                # wih on gpsimd, whh + enc tail on scalar
                for i in range(PC_N):
                    wsl = slice(i * 768, (i + 1) * 768)
                    nc.gpsimd.dma_start(out=wih_b[:, wsl], in_=wih[:, wsl])
                    csl = slice(i * R * BC, i * R * BC + 448)
                    nc.sync.dma_start(out=enc_b[:, csl], in_=encT[:, csl])
                nc.scalar.dma_start(out=whh_b, in_=whh[:, :])
                for i in range(PC_N):
                    csl = slice(i * R * BC + 448, i * R * BC + 896)
                    nc.sync.dma_start(out=enc_b[:, csl], in_=encT[:, csl])
                for i in range(PC_N):
                    csl = slice(i * R * BC + 896, (i + 1) * R * BC)
                    nc.scalar.dma_start(out=enc_b[:, csl], in_=encT[:, csl])
                enc_t = [enc_b[:, i * R * BC : (i + 1) * R * BC] for i in range(PC_N)]
                wih_t = [wih_b[:, i * 768 : (i + 1) * 768] for i in range(PC_N)]

                # gate-major gi store: gisG[g] cols = r*448 + t*224 + x,
                # with the per-(gate,t) bias folded in at drain time so the
                # sigmoid/tanh activations need no bias and t-pair into one op
                gisG = [
                    p1.tile([128, R * 448], bf16, tag=f"gis{g}", name=f"gis{g}")
                    for g in range(3)
                ]
                gi_bias = {
                    0: brz_t[:, 0:1], 1: brz_t[:, 1:2],
                    2: brz_t[:, 2:3], 3: brz_t[:, 3:4],
                    4: bihn_t[:, 0:1], 5: bihn_t[:, 1:2],
                }

                def gi_pair(g, r):
                    return gisG[g][:, r * 448 : (r + 1) * 448]

                def emit_gi_chunk(cj):
                    # full 10-pc accumulation in PSUM over a 2-r chunk; one
                    # bias-folding drain per (m, chunk), DVE/scalar alternating
                    for m in range(6):
                        g, t = m // 2, m % 2
                        ps = psGI.tile(
                            [128, 448], f32, tag="gi", name=f"gi_{cj}_{m}"
                        )
                        for pc in range(PC_N):
                            nc.tensor.matmul(
                                ps,
                                wih_t[pc][:, m * 128 : (m + 1) * 128],
                                enc_t[pc][:, cj * 448 : cj * 448 + 448],
                                start=(pc == 0),
                                stop=(pc == PC_N - 1),
                            )
                        src = ps.rearrange("p (r x) -> p r x", r=2)
                        dst = gisG[g].rearrange(
                            "p (r t x) -> p r t x", r=R, t=2
                        )[:, 2 * cj : 2 * cj + 2, t, :]
                        # all drains on scalar: DVE is the phase-1 bottleneck
                        nc.scalar.activation(
                            dst, src, Act.Identity, bias=gi_bias[m]
                        )

                def emit_gru_step(r):
                    hprev = [zb[:, h * 448 : h * 448 + 224] for h in range(HC_N)] \
                        if r == 0 else [ctx_r(h, r - 1) for h in range(HC_N)]
                    hprev_pair = zb_pair() if r == 0 else ctx_pair(r - 1)
                    ghp = []
                    for mp in range(3):
                        ps = psGH.tile([128, 512], f32, tag="gh", name=f"gh_{r}_{mp}")
                        for half in range(2):
                            m = mp * 2 + half
                            for hc in range(HC_N):
                                nc.tensor.matmul(
                                    ps[:, half * 256 : half * 256 + BC],
                                    whh_t[hc][:, m * 128 : (m + 1) * 128],
                                    hprev[hc],
                                    start=(hc == 0),
                                    stop=(hc == HC_N - 1),
                                )
                        ghp.append(ps)

                    def gh_pair(g):
                        # [128, t(2), BC] view over the gate's two t-halves
                        return ghp[g].rearrange("p (t x) -> p t x", t=2)[:, :, :BC]

                    def pv(tile):
                        return tile.rearrange("p (t x) -> p t x", t=2)

                    # t-paired gate math (448 wide)
                    tA = p1.tile([128, 448], f32, tag="tAB", bufs=2, name=f"tA{r}")
                    nc.vector.tensor_tensor(
                        pv(tA), pv(gi_pair(0, r)), gh_pair(0), op=Alu.add
                    )
                    rt = p1.tile([128, 448], bf16, tag="rt", bufs=2, name=f"rt{r}")
                    nc.scalar.activation(rt, tA, Act.Sigmoid)
                    tB = p1.tile([128, 448], f32, tag="tAB", bufs=2, name=f"tB{r}")
                    nc.vector.tensor_tensor(
                        pv(tB), pv(gi_pair(1, r)), gh_pair(1), op=Alu.add
                    )
                    zt = p1.tile([128, 448], bf16, tag="zt", bufs=2, name=f"zt{r}")
                    nc.scalar.activation(zt, tB, Act.Sigmoid)
                    tV = p1.tile([128, 448], f32, tag="tVW", bufs=2, name=f"tV{r}")
                    for t in range(2):
                        nc.vector.scalar_tensor_tensor(
                            tV[:, t * BC : (t + 1) * BC],
                            gh_pair(2)[:, t, :],
                            bhhn_t[:, t : t + 1],
                            rt[:, t * BC : (t + 1) * BC],
                            op0=Alu.add, op1=Alu.mult,
                        )
                    tW = p1.tile([128, 448], f32, tag="tVW", bufs=2, name=f"tW{r}")
                    nc.vector.tensor_tensor(tW, tV, gi_pair(2, r), op=Alu.add)
                    nt = p1.tile([128, 448], bf16, tag="nt", bufs=2, name=f"nt{r}")
                    nc.scalar.activation(nt, tW, Act.Tanh)
                    # SBUF-only fp16 tail on the (otherwise idle) gpsimd
                    tD = p1.tile([128, 448], bf16, tag="tD", bufs=2, name=f"tD{r}")
                    nc.gpsimd.tensor_tensor(pv(tD), hprev_pair, pv(nt), op=Alu.subtract)
                    tE = p1.tile([128, 448], bf16, tag="tE", bufs=2, name=f"tE{r}")
                    nc.gpsimd.tensor_tensor(tE, zt, tD, op=Alu.mult)
                    nc.gpsimd.tensor_tensor(ctx_pair(r), pv(nt), pv(tE), op=Alu.add)

                # ---- schedule: gi + GRU + pass-A preds interleaved ----
                emit_gi_chunk(0)
                emit_gru_step(0)
                emit_gi_chunk(1)
                emit_gru_step(1)
                emit_gi_chunk(2)
                psGI.release()
                wk2 = emit_wk(0, 2)
                emit_preds_chunk(0, 2, wk2, 1, 2, [0, 1])
                emit_gru_step(2)
                wk3 = emit_wk(0, 3)
                emit_preds_chunk(0, 3, wk3, 5, 2, [0, 1])
                emit_gru_step(3)
                emit_preds_chunk(0, 2, wk2, 3, 2, [2, 3])
                emit_preds_chunk(0, 3, wk3, 7, 1, [2])
                wk4 = emit_wk(0, 4)
                emit_preds_chunk(0, 4, wk4, 8, 2, [0, 1])
                emit_gru_step(4)
                wk1 = emit_wk(0, 1)
                emit_preds_chunk(0, 1, wk1, 0, 1, [0])
                emit_gru_step(5)
                # dots-phase DMAs queue behind everything front-critical
                for s in range(4):
                    sl = slice(32 * s, 32 * s + 32)
                    nc.sync.dma_start(out=encB_b[sl, :], in_=encB[sl, :])
                nc.sync.dma_start(out=cnt1_t, in_=cnt1[:, :])
                nc.sync.dma_start(out=corr_t, in_=corr[:, :])
                nc.sync.dma_start(out=posm_t, in_=posm[:, :])

            # ---- phase 3: dots + loss (pass-B preds emitted here too) ----
            psGH.release()
            psMS = tc.alloc_tile_pool(name="psMS", bufs=4, space="PSUM")
            # second preds buffer in the space phase 1 freed: pass-B clips
            # need not wait for pass-A dots to finish reading predsT
            ppB = tc.alloc_tile_pool(name="ppB", bufs=1)
            predsT_B = [
                ppB.tile([128, BS * HALF * C], bf16, tag=f"ptB{i}", name=f"ptB{i}")
                for i in range(PC_N)
            ]

            # post parts: (global group offset, group count, half); the last
            # quarter is split so less softmax work trails the final matmul
            POST_PARTS = [(0, 16, 0), (16, 16, 0), (32, 16, 1),
                          (48, 8, 1), (56, 8, 1)]

            def emit_post_part(pi):
                g0, PG, h = POST_PARTS[pi]
                lo = g0 * IJ
                hi = (g0 + PG) * IJ
                Dh = D[:, lo:hi]
                B2h = ppD.tile([70, PG * IJ], f32, tag="b2", bufs=2, name=f"b2_{pi}")
                Dv = Dh.rearrange("p (g j) -> p g j", j=IJ)
                B2v = B2h.rearrange("p (g j) -> p g j", j=IJ)
                cnt_h = cnt1_t[:, lo:hi]
                gsl = slice(g0, g0 + PG)
                mxh = mx[:, gsl]
                seh = se[:, gsl]
                posh = pos[:, gsl]
                lnvh = lnv[:, gsl]
                corrh = corr2[:, gsl]
                # dead-cell (-60000) mask is folded into corr on host
                nc.vector.tensor_reduce(mxh, Dv, axis=mybir.AxisListType.X, op=Alu.max)
                nc.gpsimd.tensor_tensor(
                    B2v, Dv, mxh.unsqueeze(2).broadcast_to([70, PG, IJ]), op=Alu.subtract
                )
                nc.scalar.activation(B2h, B2h, Act.Exp)
                nc.vector.tensor_tensor(B2h, B2h, cnt_h, op=Alu.mult)
                nc.vector.tensor_reduce(seh, B2v, axis=mybir.AxisListType.X, op=Alu.add)
                # pos = sum(D * posmask) (exact: zeros elsewhere)
                pmh = posm_t[:, h * IJ : (h + 1) * IJ]
                nc.gpsimd.tensor_tensor(
                    B2v, Dv, pmh.unsqueeze(1).broadcast_to([70, PG, IJ]), op=Alu.mult
                )
                nc.vector.tensor_reduce(posh, B2v, axis=mybir.AxisListType.X, op=Alu.add)
                # loss = ln(se) + mx - pos ; correct = (pos >= mx)
                nc.scalar.activation(lnvh, seh, Act.Ln)
                nc.vector.tensor_tensor(lnvh, lnvh, mxh, op=Alu.add)
                nc.vector.tensor_tensor(corrh, posh, mxh, op=Alu.is_ge)
                nc.vector.tensor_tensor(lnvh, lnvh, posh, op=Alu.subtract)
                nc.vector.tensor_reduce(
                    Ssum[:, 2 * pi : 2 * pi + 1], lnvh,
                    axis=mybir.AxisListType.X, op=Alu.add,
                )
                nc.vector.tensor_reduce(
                    Ssum[:, 2 * pi + 1 : 2 * pi + 2], corrh,
                    axis=mybir.AxisListType.X, op=Alu.add,
                )

            # pass-A dots start immediately; pass-B preds + posts interleave
            emit_dots_pass(0)
            wk0 = emit_wk(1, 0)
            emit_preds_chunk(1, 0, wk0, 0, 2, [0, 1])
            emit_preds_chunk(1, 0, wk0, 2, 2, [2, 3])
            emit_post_part(0)
            emit_preds_chunk(1, 0, wk0, 4, 2, [4, 5])
            wk1b = emit_wk(1, 1)
            emit_preds_chunk(1, 1, wk1b, 6, 1, [1])
            emit_preds_chunk(1, 1, wk1b, 7, 1, [2])
            emit_post_part(1)
            emit_preds_chunk(1, 1, wk1b, 8, 1, [3])
            emit_preds_chunk(1, 1, wk1b, 9, 1, [4])
            emit_dots_pass(1, 0, 16)
            emit_post_part(2)
            emit_dots_pass(1, 16, 24)
            emit_post_part(3)
            emit_dots_pass(1, 24, 32)
            emit_post_part(4)

            # combine parts: [loss, acc] = colsums of Ssum pairs
            ones = ppD.tile([70, 1], f32, tag="ones")
            nc.vector.memset(ones, 1.0)
            fp = psMS.tile([1, 10], f32, tag="dp", name="fin")
            nc.tensor.matmul(fp, ones, Ssum, start=True, stop=True)
            fs = ppD.tile([1, 10], f32, tag="fs")
            nc.vector.tensor_copy(fs, fp)
            fs2 = ppD.tile([1, 4], f32, tag="fs2")
            nc.vector.tensor_tensor(fs2[:, 0:2], fs[:, 0:2], fs[:, 2:4], op=Alu.add)
            nc.vector.tensor_tensor(fs2[:, 2:4], fs[:, 4:6], fs[:, 6:8], op=Alu.add)
            fs3 = ppD.tile([1, 2], f32, tag="fs3")
            nc.vector.tensor_tensor(fs3, fs2[:, 0:2], fs2[:, 2:4], op=Alu.add)
            nc.vector.tensor_tensor(outS, fs3, fs[:, 8:10], op=Alu.add)
            nc.sync.dma_start(out=out[:, :], in_=outS)
            psMS.release()
            psPP.release()
            ppB.release()
            ppD.release()
            ppA.release()

    nc.finalize()
    return nc


def _prep_inputs(encodings, hidden, W_ih, W_hh, b_ih, b_hh, Wk_w, Wk_b,
                 neg_rows, neg_cols):
    """Host-side reformat of the full inputs into per-core DMA-clean arrays."""
    bf16 = np.float16
    enc = np.ascontiguousarray(encodings, dtype=np.float32)
    e6 = enc.reshape(NCORE, BS, C, C, PC_N, 128)  # (core, b, i, c, pc, pp)
    # GRU layout: [core, pc, pp, r*BC + b*7 + c], r < 6
    encT = np.ascontiguousarray(
        e6[:, :, :R].transpose(0, 5, 4, 2, 1, 3)   # (core, pp, pc, r, b, c)
    ).reshape(NCORE, 128, PC_N * R * BC).astype(bf16)
    # dots layout: [core, pc, pp, b*49 + i*7 + c]
    encB = np.ascontiguousarray(
        e6.transpose(0, 5, 4, 1, 2, 3)   # (core, pp, pc, b, i, c)
    ).reshape(NCORE, 128, PC_N * BS * IJ).astype(bf16)

    wih = np.ascontiguousarray(
        W_ih.T.reshape(PC_N, 128, 768).transpose(1, 0, 2), dtype=np.float32
    ).reshape(128, PC_N * 768).astype(bf16)
    whh = np.ascontiguousarray(
        W_hh.T.reshape(HC_N, 128, 768).transpose(1, 0, 2), dtype=np.float32
    ).reshape(128, HC_N * 768).astype(bf16)
    wkh = np.ascontiguousarray(
        Wk_w.transpose(0, 2, 1).reshape(K, HC_N, 128, P).transpose(0, 2, 1, 3),
        dtype=np.float32,
    ).reshape(K, 128, HC_N * P).astype(bf16)
    bsum = (b_ih + b_hh).astype(np.float32)
    brz = np.ascontiguousarray(bsum[:512].reshape(4, 128).T)
    bihn = np.ascontiguousarray(b_ih[512:].astype(np.float32).reshape(2, 128).T)
    bhhn = np.ascontiguousarray(b_hh[512:].astype(np.float32).reshape(2, 128).T)
    wkbT = np.ascontiguousarray(
        Wk_b.astype(np.float32).reshape(K, PC_N, 128).transpose(2, 0, 1)
    ).reshape(128, K * PC_N)
    wklo = -1.0 - wkbT
    wkhi = 1.0 - wkbT
    # rank-1 bias correction: corr[k, b, ij] = sum_p Wk_b[k,p] * enc[b,i,j,p]
    corr_k = np.einsum(
        "kp,bijp->kbij", Wk_b.astype(np.float32), enc, optimize=True
    ).reshape(K, B, IJ)
    # expand to device layout [core, row=q*7+c, half, b_local, j] (k by pair)
    corr_dev = np.empty((NCORE, HALF * C, 2, BS, IJ), dtype=np.float32)
    for half in range(2):
        for qq in range(HALF):
            k, _r = PAIRS[half * HALF + qq]
            for c in range(C):
                corr_dev[:, qq * 7 + c, half] = corr_k[k].reshape(NCORE, BS, IJ)

    # negatives -> multiplicity counts over the 49 cells, plus the positive
    neg_idx = (neg_rows.astype(np.int64) * 7 + neg_cols.astype(np.int64))  # [B,K,R,C,63]
    sel = np.stack([neg_idx[:, k, r] for (k, r) in PAIRS], axis=1)  # [B,20,C,63]
    flat = (
        np.arange(B * NPAIR * C, dtype=np.int64)[:, None] * IJ
        + sel.reshape(B * NPAIR * C, S - 1)
    ).ravel()
    cnts = np.bincount(flat, minlength=B * NPAIR * C * IJ).reshape(
        B, NPAIR, C, IJ
    ).astype(np.float32)
    cvec = np.arange(C)
    for pi, (k, r) in enumerate(PAIRS):
        cnts[:, pi, cvec, r * 7 + cvec] += 1.0   # include the positive
    # device layout [core, row=q*7+c, half, b_local, j]
    cnt_dev = np.ascontiguousarray(
        cnts.reshape(NCORE, BS, 2, HALF, C, IJ).transpose(0, 3, 4, 2, 1, 5)
    )  # [core, HALF, C, 2, BS, IJ]
    cnt1 = cnt_dev.reshape(NCORE, HALF * C, 2 * BS * IJ).astype(bf16)
    # dead cells (multiplicity 0) get a large negative logit folded into
    # corr so the device softmax skips the explicit is_eq mask pass
    corr_dev = corr_dev.reshape(NCORE, HALF, C, 2, BS, IJ)
    corr_dev[cnt_dev.reshape(NCORE, HALF, C, 2, BS, IJ) == 0] = -60000.0
    corr_dev = corr_dev.reshape(NCORE, HALF * C, 2 * BS * IJ).astype(bf16)

    posm = np.zeros((HALF * C, 2, IJ), dtype=np.float32)
    for half in range(2):
        for qq in range(HALF):
            k, r = PAIRS[half * HALF + qq]
            for c in range(C):
                posm[qq * 7 + c, half, r * 7 + c] = 1.0
    posm = posm.reshape(HALF * C, 2 * IJ)

    in_maps = []
    for core in range(NCORE):
        in_maps.append(
            {
                "encT": encT[core],
                "encB": encB[core],
                "wih": wih,
                "whh": whh,
                "wk": wkh,
                "brz": brz,
                "bihn": bihn,
                "bhhn": bhhn,
                "wklo": wklo,
                "wkhi": wkhi,
                "corr": corr_dev[core],
                "cnt1": cnt1[core],
                "posm": posm,
            }
        )
    return in_maps


def _get_program():
    if "nc" not in _CACHE:
        _CACHE["nc"] = _build_program()
    return _CACHE["nc"]


def run_on_device(in_maps, trace=False, tmpdir=None):
    from concourse.bass_utils import run_bass_kernel_spmd

    nc = _get_program()
    return run_bass_kernel_spmd(
        nc, in_maps, list(range(NCORE)), trace=trace, tmpdir=tmpdir
    )


def kernel(**inputs):
    in_maps = _prep_inputs(**inputs)
    res = run_on_device(in_maps)
    loss_sum = 0.0
    corr_sum = 0.0
    for core in range(NCORE):
        o = res.results[core]["out"]
        loss_sum += float(o[0, 0])
        corr_sum += float(o[0, 1])
    loss = np.float32(loss_sum / N_PREDS)
    acc = np.float32(corr_sum / N_PREDS)
    return loss, acc
